# revision 1
# baseline (speedup 1.0000x reference)
"""Trainium2 Bass kernel for the CRAFT-style hard-negative-mining MSE loss.

Reference math (per branch, over N = 16*768*768 flat pixels):
    all_loss = (pred - target)^2
    pos_mask = (target >= 0.3) & (weight != 0)
    neg_mask = (target < 0.1)
    pos_sum  = sum(pos_mask * all_loss * weight)
    k        = min(max(1000, 3*num_pos), num_neg)
    topk_sum = sum of k largest all_loss among negatives
    loss     = (pos_sum + topk_sum) / (num_pos + k)
    out      = loss_char + loss_aff

With uniform targets num_pos ~ 0.7*N, so 3*num_pos >> num_neg and
k == num_neg: the top-k degenerates to the full sum over negatives.
The device kernel computes, per 1/8 shard, per branch:
    S1 = sum(neg_mask * all_loss)          (DVE scalar_tensor_tensor+accum)
    S2 = sum((t>=0.3) * all_loss * weight) (DVE, (w!=0) is absorbed by *w)
    sumsign_neg = sum(sign(0.1 - t))       (ScalarE Sign activation+accum)
    sumsign_pos = sum(sign(t - 0.3))       (ScalarE Sign activation+accum)
Counts follow exactly from the sign sums (thresholds 0.1f/0.3f are not
bf16-representable, so sign is never 0). The host merges the 8 shards and
applies the k/denominator logic; a full numpy fallback covers the
(never-hit-here) k < num_neg case.

Inputs are cast to bf16 on the host: halves HBM traffic and doubles DVE
throughput; measured end-to-end relative error is ~1e-4 (dominated by
threshold reclassification of ~0.05% of pixels near 0.1/0.3).

All six shard tensors are packed into one [P, 6, FD] DRAM tensor per core
so each tile iteration needs a single DMA (instructions on trn2 have very
few semaphore-wait slots; one DMA semaphore per iteration keeps every
consumer at <=1 wait; Bacc.compile()'s generate_event_semaphores splits
the rest).
"""

import os
import numpy as np
import ml_dtypes

N_CORES = 8
B, H, W = 16, 768, 768
NPX = B * H * W              # 9_437_184 flat pixels
P = 128                      # SBUF partitions
FD = NPX // (N_CORES * P)    # 9216 free-dim elements per core per tensor
N_TILES = 4                  # tiles per branch
F = FD // N_TILES            # tile width

USE_BF16 = os.environ.get("KERNEL_FP32", "0") != "1"

THRESH_NEG = 0.1
THRESH_POS = 0.3

# quantity indices in the accumulators
Q_S1, Q_S2 = 0, 1            # DVE accumulator columns
Q_SGN_NEG, Q_SGN_POS = 0, 1  # ACT accumulator columns

_compiled = None             # cached nc
LAST_RESULTS = None          # BassKernelResults of the last run (for profiling)


def _build_nc():
    import concourse.bacc as bacc
    import concourse.mybir as mybir
    import concourse.tile as tile
    from contextlib import ExitStack

    DT = mybir.dt.bfloat16 if USE_BF16 else mybir.dt.float32
    f32 = mybir.dt.float32
    Alu = mybir.AluOpType
    Act = mybir.ActivationFunctionType

    nc = bacc.Bacc(
        "TRN2",
        target_bir_lowering=False,
        debug=False,
        num_devices=N_CORES,
    )

    # bias constants for the Sign activations, registered pre-Tile like
    # Bass's own const APs (memset + barrier; no Tile-tracked deps)
    bias_neg_t = nc.alloc_sbuf_tensor("bias_neg_c", [P, 1], f32)
    nc.gpsimd.memset(bias_neg_t.ap(), THRESH_NEG)
    bias_pos_t = nc.alloc_sbuf_tensor("bias_pos_c", [P, 1], f32)
    nc.gpsimd.memset(bias_pos_t.ap(), -THRESH_POS)
    nc.all_engine_barrier()
    bias_neg = bias_neg_t.ap()
    bias_pos = bias_pos_t.ap()

    # packed input: dim1 = (p_c, t_c, w_c, p_a, t_a, w_a)
    pk = nc.declare_dram_parameter("pk", [P, 6, FD], DT, isOutput=False)
    out_dve = nc.declare_dram_parameter("acc_dve", [P, 2 * 2 * N_TILES], f32, isOutput=True)
    out_act = nc.declare_dram_parameter("acc_act", [P, 2 * 2 * N_TILES], f32, isOutput=True)

    with tile.TileContext(nc) as tc, ExitStack() as ctx:
        in_pool = ctx.enter_context(tc.tile_pool(name="in", bufs=3))
        tmp_pool = ctx.enter_context(tc.tile_pool(name="tmp", bufs=2))
        acc_pool = ctx.enter_context(tc.tile_pool(name="acc", bufs=1))

        acc_dve = acc_pool.tile([P, 2 * 2 * N_TILES], f32, tag="acc_dve")
        acc_act = acc_pool.tile([P, 2 * 2 * N_TILES], f32, tag="acc_act")

        for b in range(2):
            for i in range(N_TILES):
                sl = slice(i * F, (i + 1) * F)
                tin = in_pool.tile([P, 3, F], DT, tag="in")
                nc.sync.dma_start(tin[:], pk[:, 3 * b : 3 * b + 3, sl])
                pt = tin[:, 0, :]
                tt = tin[:, 1, :]
                wt = tin[:, 2, :]

                def dcol(q):
                    j = (b * 2 + q) * N_TILES + i
                    return acc_dve[:, j : j + 1]

                def acol(q):
                    j = (b * 2 + q) * N_TILES + i
                    return acc_act[:, j : j + 1]

                # d = pred - target            (DVE)
                d = tmp_pool.tile([P, F], DT, tag="d")
                nc.vector.tensor_tensor(d[:], pt, tt, Alu.subtract)
                # l = d^2                      (ScalarE)
                l = tmp_pool.tile([P, F], DT, tag="l")
                nc.scalar.activation(l[:], d[:], Act.Square)
                # lw = l * w                   (DVE)
                lw = tmp_pool.tile([P, F], DT, tag="lw")
                nc.vector.tensor_tensor(lw[:], l[:], wt, Alu.mult)
                # S1 += sum((t < 0.1) * l)     (DVE fused mask+mul+reduce)
                scr_d = tmp_pool.tile([P, F], DT, tag="scr_d")
                nc.vector.scalar_tensor_tensor(
                    scr_d[:], tt, THRESH_NEG, l[:], Alu.is_lt, Alu.mult,
                    accum_out=dcol(Q_S1),
                )
                # S2 += sum((t >= 0.3) * l * w)
                nc.vector.scalar_tensor_tensor(
                    scr_d[:], tt, THRESH_POS, lw[:], Alu.is_ge, Alu.mult,
                    accum_out=dcol(Q_S2),
                )
                # sumsign_neg += sum(sign(0.1 - t))   (ScalarE)
                scr_a = tmp_pool.tile([P, F], DT, tag="scr_a")
                nc.scalar.activation(
                    scr_a[:], tt, Act.Sign, bias=bias_neg, scale=-1.0,
                    accum_out=acol(Q_SGN_NEG),
                )
                # sumsign_pos += sum(sign(t - 0.3))   (ScalarE)
                nc.scalar.activation(
                    scr_a[:], tt, Act.Sign, bias=bias_pos, scale=1.0,
                    accum_out=acol(Q_SGN_POS),
                )

        nc.sync.dma_start(out_dve[:], acc_dve[:])
        nc.sync.dma_start(out_act[:], acc_act[:])

    nc.compile()
    return nc


def _get_nc():
    global _compiled
    if _compiled is None:
        _compiled = _build_nc()
    return _compiled


def _np_branch_fallback(pred, target, weight):
    """Exact reference math in numpy float64 (handles k < num_neg)."""
    pred = pred.astype(np.float64)
    target = target.astype(np.float64)
    weight = weight.astype(np.float64)
    all_loss = (pred - target) ** 2
    pos_mask = (target >= THRESH_POS) & (weight != 0)
    neg_mask = target < THRESH_NEG
    pos_sum = float(np.sum(np.where(pos_mask, all_loss * weight, 0.0)))
    num_pos = int(np.sum(pos_mask))
    num_neg = int(np.sum(neg_mask))
    k = min(max(1000, 3 * num_pos), num_neg)
    neg_vals = all_loss[neg_mask]
    if k >= num_neg:
        topk = float(neg_vals.sum())
    elif k <= 0:
        topk = 0.0
    else:
        topk = float(np.partition(neg_vals, num_neg - k)[num_neg - k :].sum())
    return (pos_sum + topk) / (num_pos + k)


def kernel(output, character_map, affinity_map, character_weight, affinity_weight):
    from concourse.bass_utils import run_bass_kernel_spmd

    global LAST_RESULTS
    np_dt = ml_dtypes.bfloat16 if USE_BF16 else np.float32

    output = np.asarray(output, dtype=np.float32)

    def shard(a):
        # flat pixel order (b, h, w) -> [core, partition, free]
        return np.ascontiguousarray(a).reshape(N_CORES, P, FD).astype(np_dt)

    packed = np.empty((N_CORES, P, 6, FD), dtype=np_dt)
    packed[:, :, 0] = shard(output[:, 0])
    packed[:, :, 1] = shard(np.asarray(character_map, dtype=np.float32))
    packed[:, :, 2] = shard(np.asarray(character_weight, dtype=np.float32))
    packed[:, :, 3] = shard(output[:, 1])
    packed[:, :, 4] = shard(np.asarray(affinity_map, dtype=np.float32))
    packed[:, :, 5] = shard(np.asarray(affinity_weight, dtype=np.float32))

    in_maps = [{"pk": packed[c]} for c in range(N_CORES)]

    nc = _get_nc()
    res = run_bass_kernel_spmd(
        nc,
        in_maps,
        list(range(N_CORES)),
        trace=os.environ.get("KERNEL_TRACE", "0") == "1",
    )
    LAST_RESULTS = res

    # [cores, P, branch, quantity, tile] -> sum over cores, partitions, tiles
    acc_dve = np.stack([r["acc_dve"] for r in res.results]).astype(np.float64)
    acc_act = np.stack([r["acc_act"] for r in res.results]).astype(np.float64)
    sums_dve = acc_dve.reshape(N_CORES, P, 2, 2, N_TILES).sum(axis=(0, 1, 4))
    sums_act = acc_act.reshape(N_CORES, P, 2, 2, N_TILES).sum(axis=(0, 1, 4))

    total = 0.0
    for bidx, (tmap, wmap) in enumerate(
        [(character_map, character_weight), (affinity_map, affinity_weight)]
    ):
        s1 = sums_dve[bidx, Q_S1]
        s2 = sums_dve[bidx, Q_S2]
        num_neg = int(round((sums_act[bidx, Q_SGN_NEG] + NPX) / 2))
        num_pos = int(round((sums_act[bidx, Q_SGN_POS] + NPX) / 2))
        k = min(max(1000, 3 * num_pos), num_neg)
        if k == num_neg:
            total += (s2 + s1) / (num_pos + k)
        else:
            # top-k actually selective: fall back to exact host computation
            total += _np_branch_fallback(
                output[:, bidx].reshape(-1),
                np.asarray(tmap, dtype=np.float32).reshape(-1),
                np.asarray(wmap, dtype=np.float32).reshape(-1),
            )

    return np.float32(total)



# revision 3
# speedup vs baseline: 1.2992x; 1.2992x over previous
"""Trainium2 Bass kernel for the CRAFT-style hard-negative-mining MSE loss.

Reference math (per branch, over N = 16*768*768 flat pixels):
    all_loss = (pred - target)^2
    pos_mask = (target >= 0.3) & (weight != 0)
    neg_mask = (target < 0.1)
    pos_sum  = sum(pos_mask * all_loss * weight)
    k        = min(max(1000, 3*num_pos), num_neg)
    topk_sum = sum of k largest all_loss among negatives
    loss     = (pos_sum + topk_sum) / (num_pos + k)
    out      = loss_char + loss_aff

With uniform targets num_pos ~ 0.7*N so k == num_neg: the top-k
degenerates to the full sum over negatives, and only S1+S2 =
sum(neg_mask*l) + sum(pos_mask*w*l) plus the two counts are needed.

v2 engine split (v1 was DVE+ACT-bound: stt ops ran in 1x DVE mode):
    DVE   : d = p - t          (tensor_tensor, 2x mode)
            u = l * w          (tensor_tensor, 2x)
            m- = (t < 0.1)     (tensor_scalar, 4x)
            m+ = (t >= 0.3)    (tensor_scalar, 4x)
    ACT   : l = Square(d)
    PE    : masked reductions via the diagonal-accumulation trick:
            for each 128-column block, matmul(stationary=mask block,
            moving=[vals | 1 | 0]) accumulated into one [128,130] PSUM
            tile per branch. diag = S1+S2 partials, col 128 = num_neg
            partials, col 129 = num_pos partials.
    Host  : trace + count merge across 8 cores, k/denominator logic.

The u = l*w op is software-pipelined one tile back so DVE never waits
on ACT's Square.
"""

import os
import numpy as np
import ml_dtypes

N_CORES = 8
B, H, W = 16, 768, 768
NPX = B * H * W              # 9_437_184 flat pixels
P = 128                      # SBUF partitions
FD = NPX // (N_CORES * P)    # 9216 free-dim elements per core per tensor
N_TILES = 4                  # tiles per branch
F = FD // N_TILES            # tile width (2304)
NBLK = F // P                # 128-column blocks per tile (18)
MW = P + 2                   # moving width: 128 values + negcnt + poscnt col

USE_BF16 = os.environ.get("KERNEL_FP32", "0") != "1"

THRESH_NEG = 0.1
THRESH_POS = 0.3

_compiled = None             # cached nc
LAST_RESULTS = None          # BassKernelResults of the last run (for profiling)


def _build_nc():
    import concourse.bacc as bacc
    import concourse.mybir as mybir
    import concourse.tile as tile
    from contextlib import ExitStack

    DT = mybir.dt.bfloat16 if USE_BF16 else mybir.dt.float32
    f32 = mybir.dt.float32
    Alu = mybir.AluOpType
    Act = mybir.ActivationFunctionType

    nc = bacc.Bacc(
        "TRN2",
        target_bir_lowering=False,
        debug=False,
        num_devices=N_CORES,
    )

    # packed input: dim1 = (p_c, t_c, w_c, p_a, t_a, w_a)
    pk = nc.declare_dram_parameter("pk", [P, 6, FD], DT, isOutput=False)
    out_acc = nc.declare_dram_parameter("acc", [P, 2, MW], f32, isOutput=True)

    with tile.TileContext(nc) as tc, ExitStack() as ctx:
        in_pool = ctx.enter_context(tc.tile_pool(name="in", bufs=3))
        d_pool = ctx.enter_context(tc.tile_pool(name="d", bufs=2))
        mv_pool = ctx.enter_context(tc.tile_pool(name="mv", bufs=2))
        msk_pool = ctx.enter_context(tc.tile_pool(name="msk", bufs=2))
        res_pool = ctx.enter_context(tc.tile_pool(name="res", bufs=1))
        ps_pool = ctx.enter_context(tc.psum_pool(name="ps", bufs=1))

        acc = [
            ps_pool.tile([P, MW], f32, tag="acc0", name="acc0"),
            ps_pool.tile([P, MW], f32, tag="acc1", name="acc1"),
        ]
        res = res_pool.tile([P, 2, MW], f32, tag="res")

        # moving tiles carry two constant columns per 128-block:
        # col 128 -> counts num_neg (1.0 in l tiles, 0.0 in u tiles)
        # col 129 -> counts num_pos (0.0 in l tiles, 1.0 in u tiles)
        for _ in range(2):
            lt = mv_pool.tile([P, NBLK, MW], DT, tag="l")
            nc.vector.memset(lt[:, :, P : P + 1], 1.0)
            nc.vector.memset(lt[:, :, P + 1 : P + 2], 0.0)
            ut = mv_pool.tile([P, NBLK, MW], DT, tag="u")
            nc.vector.memset(ut[:, :, P : P + 1], 0.0)
            nc.vector.memset(ut[:, :, P + 1 : P + 2], 1.0)

        NT = 2 * N_TILES  # global tile count
        prev = None       # (b, tin, lt, ut, mneg, mpos) of tile g-1

        for g in range(NT + 1):
            if g < NT:
                b, i = divmod(g, N_TILES)
                sl = slice(i * F, (i + 1) * F)
                tin = in_pool.tile([P, 3, NBLK, P], DT, tag="in")
                nc.sync.dma_start(tin[:], pk[:, 3 * b : 3 * b + 3, sl])

            # software-pipelined tail of tile g-1: u = l * w, then its
            # 36 matmuls (mask blocks x [vals|counts]) on the PE.
            if prev is not None:
                pb, ptin, plt, put, pmneg, pmpos = prev
                nc.vector.tensor_tensor(
                    put[:, :, 0:P], plt[:, :, 0:P], ptin[:, 2], Alu.mult
                )
                first = (pb, (g - 1) % N_TILES) == (pb, 0) and True
                for k in range(NBLK):
                    is_first = (g - 1) % N_TILES == 0 and k == 0
                    nc.tensor.matmul(
                        acc[pb][:],
                        pmneg[:, k, :],
                        plt[:, k, :],
                        start=is_first,
                        stop=False,
                    )
                    is_last = (g - 1) % N_TILES == N_TILES - 1 and k == NBLK - 1
                    nc.tensor.matmul(
                        acc[pb][:],
                        pmpos[:, k, :],
                        put[:, k, :],
                        start=False,
                        stop=is_last,
                    )
                if (g - 1) % N_TILES == N_TILES - 1:
                    # branch pb finished: move PSUM accumulator to SBUF
                    nc.vector.tensor_copy(res[:, pb, :], acc[pb][:])

            if g < NT:
                pt = tin[:, 0]
                tt = tin[:, 1]
                d = d_pool.tile([P, NBLK, P], DT, tag="d")
                nc.vector.tensor_tensor(d[:], pt, tt, Alu.subtract)
                mneg = msk_pool.tile([P, NBLK, P], DT, tag="mneg")
                nc.vector.tensor_scalar(mneg[:], tt, THRESH_NEG, None, Alu.is_lt)
                mpos = msk_pool.tile([P, NBLK, P], DT, tag="mpos")
                nc.vector.tensor_scalar(mpos[:], tt, THRESH_POS, None, Alu.is_ge)
                lt = mv_pool.tile([P, NBLK, MW], DT, tag="l")
                nc.scalar.activation(lt[:, :, 0:P], d[:], Act.Square)
                ut = mv_pool.tile([P, NBLK, MW], DT, tag="u")
                prev = (b, tin, lt, ut, mneg, mpos)

        nc.sync.dma_start(out_acc[:], res[:])

    nc.compile()
    return nc


def _get_nc():
    global _compiled
    if _compiled is None:
        _compiled = _build_nc()
    return _compiled


def _np_branch_fallback(pred, target, weight):
    """Exact reference math in numpy float64 (handles k < num_neg)."""
    pred = pred.astype(np.float64)
    target = target.astype(np.float64)
    weight = weight.astype(np.float64)
    all_loss = (pred - target) ** 2
    pos_mask = (target >= THRESH_POS) & (weight != 0)
    neg_mask = target < THRESH_NEG
    pos_sum = float(np.sum(np.where(pos_mask, all_loss * weight, 0.0)))
    num_pos = int(np.sum(pos_mask))
    num_neg = int(np.sum(neg_mask))
    k = min(max(1000, 3 * num_pos), num_neg)
    neg_vals = all_loss[neg_mask]
    if k >= num_neg:
        topk = float(neg_vals.sum())
    elif k <= 0:
        topk = 0.0
    else:
        topk = float(np.partition(neg_vals, num_neg - k)[num_neg - k :].sum())
    return (pos_sum + topk) / (num_pos + k)


def kernel(output, character_map, affinity_map, character_weight, affinity_weight):
    from concourse.bass_utils import run_bass_kernel_spmd

    global LAST_RESULTS
    np_dt = ml_dtypes.bfloat16 if USE_BF16 else np.float32

    output = np.asarray(output, dtype=np.float32)

    def shard(a):
        # flat pixel order (b, h, w) -> [core, partition, free]
        return np.ascontiguousarray(a).reshape(N_CORES, P, FD).astype(np_dt)

    packed = np.empty((N_CORES, P, 6, FD), dtype=np_dt)
    packed[:, :, 0] = shard(output[:, 0])
    packed[:, :, 1] = shard(np.asarray(character_map, dtype=np.float32))
    packed[:, :, 2] = shard(np.asarray(character_weight, dtype=np.float32))
    packed[:, :, 3] = shard(output[:, 1])
    packed[:, :, 4] = shard(np.asarray(affinity_map, dtype=np.float32))
    packed[:, :, 5] = shard(np.asarray(affinity_weight, dtype=np.float32))

    in_maps = [{"pk": packed[c]} for c in range(N_CORES)]

    nc = _get_nc()
    res = run_bass_kernel_spmd(
        nc,
        in_maps,
        list(range(N_CORES)),
        trace=os.environ.get("KERNEL_TRACE", "0") == "1",
    )
    LAST_RESULTS = res

    # acc: [cores, P, branch, MW]; diag over [:,:128] = S1+S2 partials,
    # col 128 = num_neg partials, col 129 = num_pos partials
    acc = np.stack([r["acc"] for r in res.results]).astype(np.float64)

    total = 0.0
    for bidx, (tmap, wmap) in enumerate(
        [(character_map, character_weight), (affinity_map, affinity_weight)]
    ):
        a = acc[:, :, bidx, :]                    # [cores, 128, 130]
        s_combined = np.trace(a[:, :, :P], axis1=1, axis2=2).sum()
        num_neg = int(round(a[:, :, P].sum()))
        num_pos = int(round(a[:, :, P + 1].sum()))
        k = min(max(1000, 3 * num_pos), num_neg)
        if k == num_neg:
            total += s_combined / (num_pos + k)
        else:
            # top-k actually selective: fall back to exact host computation
            total += _np_branch_fallback(
                output[:, bidx].reshape(-1),
                np.asarray(tmap, dtype=np.float32).reshape(-1),
                np.asarray(wmap, dtype=np.float32).reshape(-1),
            )

    return np.float32(total)
